# revision 6
# baseline (speedup 1.0000x reference)
"""MoE projection layer (4 heterogeneous experts, top-2 routing) on 8 TRN2 cores.

Strategy: data-parallel over tokens with host-side routing.
  - Host: gate softmax + top-2 + renormalize (exact replica of the reference
    math), gather each expert's tokens, shard them across the 8 cores, pad to
    a fixed capacity.
  - Device (SPMD, no collectives): each core runs all 4 expert MLPs on its
    token shards. Activations are kept feature-major [D, tokens] in bf16 so
    every layer is a chain of TensorE matmuls (lhsT = weight slab, rhs =
    activation). LayerNorm is handled without cross-partition reductions:
      stats  : ones-vector matmuls accumulate sum(a) and sum(a^2) in PSUM
      center : folded into the next matmul as an appended K-row
               (lhsT row = colsum(W), rhs row = -mu)
      scale  : rstd broadcast across partitions via a PE outer product, then
               one VectorE multiply on the next layer's PSUM accumulation
  - Host: transpose back, multiply by gate weights, scatter-add in fp32.
"""
import numpy as np
import ml_dtypes

import concourse.bass as bass
import concourse.tile as tile
from concourse import mybir
from concourse.bass_utils import run_bass_kernel_spmd

P = 128
EPS = 1e-5
D_MODEL = 1024
NUM_EXPERTS = 4
TOP_K = 2
N_CORES = 8
ACTS = ["gelu", "silu", "relu", "leaky_relu"]
NB_MAX = 640  # max tokens per device block (PSUM bank budget)

BF = ml_dtypes.bfloat16
bf16d = mybir.dt.bfloat16
f32 = mybir.dt.float32

ACT_FN = {
    "gelu": mybir.ActivationFunctionType.Gelu,
    "silu": mybir.ActivationFunctionType.Silu,
    "relu": mybir.ActivationFunctionType.Relu,
    "leaky_relu": mybir.ActivationFunctionType.Lrelu,
}
ACT_ALPHA = {"leaky_relu": 0.01}

_prog_cache = {}


def _split_excess_waits(nc, max_waits=1):
    """This walrus rejects >1 sem wait on one CTRL instruction; hoist extras
    onto spliced InstDrain copies placed just before the offender."""
    n_new = 0
    for f in nc.m.functions:
        for b in f.blocks:
            out, changed = [], False
            for inst in b.instructions:
                si = inst.sync_info
                waits = list(si.on_wait) if (si and si.on_wait) else []
                if len(waits) > max_waits:
                    changed = True
                    extra = waits[: len(waits) - max_waits]
                    keep = waits[len(waits) - max_waits:]
                    while extra:
                        chunk, extra = extra[:max_waits], extra[max_waits:]
                        d = mybir.InstDrain(
                            name=f"I-splitw-{n_new}", ins=[], outs=[],
                            bass_is_fusable=False)
                        d.engine = inst.engine
                        d.sync_info = mybir.SyncInfo(on_wait=chunk, on_update=[])
                        out.append(d)
                        n_new += 1
                    inst.sync_info = mybir.SyncInfo(
                        on_wait=keep, on_update=list(si.on_update))
                out.append(inst)
            if changed:
                b.instructions = out
    return n_new


def _prep_layers(expert_params):
    """Fold LN affine params into weights; return per-expert matmul layer
    descriptors (host numpy)."""
    experts = []
    for e in range(NUM_EXPERTS):
        p = expert_params[e]
        layers = []
        g_prev = None
        beta_prev = None
        seq = [tuple(np.asarray(a, np.float32) for a in hl) for hl in p["hidden"]]
        Wo, bo, go, bto = (np.asarray(a, np.float32) for a in p["out"])
        for li, (W, b, g, beta) in enumerate(seq):
            if li == 0:
                Weff, bias = W, b.copy()
            else:
                Weff = g_prev[:, None] * W
                bias = beta_prev @ W + b
            layers.append(dict(W=Weff, bias=bias, act=ACTS[e], fold=li > 0))
            g_prev, beta_prev = g, beta
        Weff = g_prev[:, None] * Wo
        bias = beta_prev @ Wo + bo
        layers.append(dict(W=Weff, bias=bias, act=None, fold=True))
        for L in layers:
            W = L["W"]
            d_in, d_out = W.shape
            KC, MC = d_in // P, d_out // P
            Wb = W.astype(BF)
            L["wb"] = np.ascontiguousarray(
                Wb.reshape(KC, P, MC, P).transpose(2, 1, 0, 3))
            L["cs"] = (Wb.astype(np.float32).sum(0)[None].astype(BF)
                       if L["fold"] else None)
            L["kc"], L["mc"] = KC, MC
            L["has_bias"] = bool(np.any(L["bias"] != 0.0))
            L["bias_col"] = (np.ascontiguousarray(
                L["bias"].reshape(MC, P).T.astype(np.float32))
                if L["has_bias"] else None)
        out_affine = (not np.allclose(go, 1.0)) or (not np.allclose(bto, 0.0))
        experts.append(dict(layers=layers, out_affine=out_affine,
                            go=go, bto=bto))
    return experts


def _build_program(caps, meta):
    """meta: per expert list of (kc, mc, fold, has_bias, act, out_affine)."""
    nc = bass.Bass()
    xin, yout, wdecl = [], [], []
    for e in range(NUM_EXPERTS):
        xin.append(nc.declare_dram_parameter(
            f"x{e}", [D_MODEL, caps[e]], bf16d, isOutput=False))
        decls = []
        for li, (kc, mc, fold, has_bias, act, out_aff) in enumerate(meta[e]):
            d = {}
            d["w"] = nc.declare_dram_parameter(
                f"w{e}_{li}", [mc, P, kc, P], bf16d, isOutput=False)
            if fold:
                d["cs"] = nc.declare_dram_parameter(
                    f"cs{e}_{li}", [1, mc * P], bf16d, isOutput=False)
            if has_bias:
                d["bias"] = nc.declare_dram_parameter(
                    f"b{e}_{li}", [P, mc], f32, isOutput=False)
            decls.append(d)
        if meta[e][-1][5]:  # out_affine
            decls.append({
                "go": nc.declare_dram_parameter(f"go{e}", [P, 8], f32,
                                                isOutput=False),
                "bto": nc.declare_dram_parameter(f"bto{e}", [P, 8], f32,
                                                 isOutput=False)})
        wdecl.append(decls)
        yout.append(nc.declare_dram_parameter(
            f"y{e}", [D_MODEL, caps[e]], f32, isOutput=True))

    cmax = min(NB_MAX, max(caps))
    pp_bufs = 3 if cmax <= 512 else 2

    with tile.TileContext(nc) as tc:
        with (
            tc.tile_pool(name="a", bufs=2) as pa,
            tc.tile_pool(name="w", bufs=2) as pw,
            tc.tile_pool(name="sq", bufs=2) as psq,
            tc.tile_pool(name="small", bufs=8) as psm,
            tc.tile_pool(name="bc", bufs=3) as pb,
            tc.tile_pool(name="yo", bufs=2) as py,
            tc.tile_pool(name="cs", bufs=1) as pcs,
            tc.tile_pool(name="const", bufs=1) as pc,
            tc.tile_pool(name="pp", bufs=pp_bufs, space="PSUM") as pp,
            tc.tile_pool(name="pps", bufs=1, space="PSUM") as pps,
            tc.tile_pool(name="ppb", bufs=1, space="PSUM") as ppb,
        ):
            ones_bf = pc.tile([P, 1], bf16d)
            nc.vector.memset(ones_bf, 1.0)
            ones_f = pc.tile([P, 1], f32)
            nc.vector.memset(ones_f, 1.0)
            ones_row = pc.tile([1, P], f32)
            nc.vector.memset(ones_row, 1.0)
            eps_t = pc.tile([1, 1], f32)
            nc.vector.memset(eps_t, EPS)

            def bcast(src_tile, nb):
                pbt = ppb.tile([P, nb], f32)
                for ns in range(0, nb, 512):
                    ne = min(ns + 512, nb)
                    nc.tensor.matmul(pbt[:, ns:ne], ones_row[0:1, :],
                                     src_tile[0:1, ns:ne],
                                     start=True, stop=True)
                out = pb.tile([P, nb], f32)
                nc.vector.tensor_copy(out, pbt)
                return out

            for e in range(NUM_EXPERTS):
                C = caps[e]
                layers = meta[e]
                xv = xin[e].rearrange("(kc p) c -> p kc c", p=P)
                yv = yout[e].rearrange("(mc p) c -> mc p c", p=P)
                go_sb = bto_sb = None
                if layers[-1][5]:  # out_affine
                    ga = wdecl[e][-1]
                    go_sb = pc.tile([P, 8], f32, tag=f"go{e}")
                    nc.gpsimd.dma_start(out=go_sb, in_=ga["go"][:, :])
                    bto_sb = pc.tile([P, 8], f32, tag=f"bto{e}")
                    nc.gpsimd.dma_start(out=bto_sb, in_=ga["bto"][:, :])
                for s0 in range(0, C, NB_MAX):
                    NB = min(NB_MAX, C - s0)
                    nchunks = [(ns, min(ns + 512, NB))
                               for ns in range(0, NB, 512)]
                    a_prev = pa.tile([P, D_MODEL // P, NB], bf16d, tag="act")
                    nc.gpsimd.dma_start(out=a_prev,
                                        in_=xv[:, :, s0:s0 + NB])
                    negmu = rstd_b = mu_f = None
                    for li, (KC, MC, fold, has_bias, act, _oa) in enumerate(layers):
                        is_out = li == len(layers) - 1
                        D = MC * P
                        dt_a = f32 if is_out else bf16d
                        a_out = pa.tile([P, MC, NB], dt_a, tag="act")
                        stats = pps.tile([33, NB], f32)
                        cs_sb = None
                        if fold:
                            cs_sb = pcs.tile([1, D], bf16d)
                            nc.gpsimd.dma_start(out=cs_sb,
                                                in_=wdecl[e][li]["cs"][:, :])
                        bias_sb = None
                        if has_bias:
                            bias_sb = psm.tile([P, MC], f32, tag="sm")
                            nc.gpsimd.dma_start(
                                out=bias_sb, in_=wdecl[e][li]["bias"][:, :])
                        for mc in range(MC):
                            wslab = pw.tile([P, KC, P], bf16d)
                            nc.gpsimd.dma_start(out=wslab,
                                                in_=wdecl[e][li]["w"][mc])
                            ps = pp.tile([P, NB], f32)
                            for (ns, ne) in nchunks:
                                for kc in range(KC):
                                    nc.tensor.matmul(
                                        ps[:, ns:ne], wslab[:, kc, :],
                                        a_prev[:, kc, ns:ne],
                                        start=(kc == 0),
                                        stop=(kc == KC - 1 and not fold))
                                if fold:
                                    nc.tensor.matmul(
                                        ps[:, ns:ne],
                                        cs_sb[0:1, mc * P:(mc + 1) * P],
                                        negmu[0:1, ns:ne],
                                        start=False, stop=True)
                            if fold:
                                nc.vector.tensor_mul(ps, ps, rstd_b)
                            bias_ap = (bias_sb[:, mc:mc + 1]
                                       if has_bias else None)
                            if is_out:
                                if bias_ap is not None:
                                    nc.scalar.activation(
                                        a_out[:, mc, :], ps,
                                        mybir.ActivationFunctionType.Identity,
                                        bias=bias_ap)
                                else:
                                    nc.scalar.copy(a_out[:, mc, :], ps)
                            else:
                                nc.scalar.activation(
                                    a_out[:, mc, :], ps, ACT_FN[act],
                                    bias=bias_ap if bias_ap is not None else 0.0,
                                    alpha=ACT_ALPHA.get(act, 0.0))
                            sq = psq.tile([P, NB], dt_a)
                            nc.vector.tensor_mul(sq, a_out[:, mc, :],
                                                 a_out[:, mc, :])
                            ones_t = ones_f if is_out else ones_bf
                            for (ns, ne) in nchunks:
                                nc.tensor.matmul(
                                    stats[0:1, ns:ne], ones_t[:, 0:1],
                                    a_out[:, mc, ns:ne],
                                    start=(mc == 0), stop=(mc == MC - 1),
                                    skip_group_check=True)
                                nc.tensor.matmul(
                                    stats[32:33, ns:ne], ones_t[:, 0:1],
                                    sq[:, ns:ne],
                                    start=(mc == 0), stop=(mc == MC - 1),
                                    skip_group_check=True)
                        mu = psm.tile([1, NB], f32, tag="sm")
                        nc.scalar.mul(mu, stats[0:1, :], 1.0 / D)
                        e2 = psm.tile([1, NB], f32, tag="sm")
                        nc.scalar.mul(e2, stats[32:33, :], 1.0 / D)
                        var = psm.tile([1, NB], f32, tag="sm")
                        nc.vector.tensor_mul(var, mu, mu)
                        nc.vector.tensor_sub(var, e2, var)
                        se = psm.tile([1, NB], f32, tag="sm")
                        nc.scalar.activation(
                            se, var, mybir.ActivationFunctionType.Sqrt,
                            bias=eps_t[0:1, 0:1])
                        rstd = psm.tile([1, NB], f32, tag="sm")
                        nc.vector.reciprocal(rstd, se)
                        rstd_b = bcast(rstd, NB)
                        if not is_out:
                            negmu = psm.tile([1, NB], bf16d, tag="sm")
                            nc.scalar.mul(negmu, mu, -1.0)
                        mu_f = mu
                        a_prev = a_out
                    # final LN apply: y = (z - mu) * rstd
                    mu_b = bcast(mu_f, NB)
                    for mc in range(D_MODEL // P):
                        tmp = psq.tile([P, NB], f32)
                        nc.vector.tensor_sub(tmp, a_prev[:, mc, :], mu_b)
                        yt = py.tile([P, NB], f32)
                        nc.vector.tensor_mul(yt, tmp, rstd_b)
                        if layers[-1][5]:  # out_affine
                            nc.vector.tensor_scalar(
                                yt, yt, go_sb[:, mc:mc + 1],
                                bto_sb[:, mc:mc + 1],
                                mybir.AluOpType.mult, mybir.AluOpType.add)
                        nc.gpsimd.dma_start(out=yv[mc, :, s0:s0 + NB],
                                            in_=yt)
    _split_excess_waits(nc)
    return nc


def kernel(x, expert_params, gate_w, gate_b):
    x = np.asarray(x, dtype=np.float32)
    gate_w = np.asarray(gate_w, np.float32)
    gate_b = np.asarray(gate_b, np.float32)
    B = x.shape[0]

    # ---- gating (host, replicates reference math) ----
    logits = x @ gate_w + gate_b
    m = logits.max(axis=-1, keepdims=True)
    ex = np.exp(logits - m)
    scores = ex / ex.sum(axis=-1, keepdims=True)
    order = np.argsort(-scores, axis=-1, kind="stable")
    top = order[:, :TOP_K]
    mask = np.zeros_like(scores)
    np.put_along_axis(mask, top, 1.0, axis=-1)
    masked = scores * mask
    gates = masked / (masked.sum(axis=-1, keepdims=True) + 1e-9)  # [B, E]

    # ---- routing: per expert, shard token list across cores ----
    chunks = []
    caps = []
    for e in range(NUM_EXPERTS):
        ids = np.nonzero(mask[:, e])[0]
        ch = np.array_split(ids, N_CORES)
        chunks.append(ch)
        mx = max(1, max(len(c) for c in ch))
        caps.append(((mx + P - 1) // P) * P)
    caps = tuple(caps)

    # ---- weights (shared across cores) ----
    experts = _prep_layers(expert_params)
    meta = tuple(
        tuple((L["kc"], L["mc"], L["fold"], L["has_bias"], L["act"],
               experts[e]["out_affine"])
              for L in experts[e]["layers"])
        for e in range(NUM_EXPERTS))

    key = (caps, meta)
    if key not in _prog_cache:
        _prog_cache[key] = _build_program(caps, meta)
    nc = _prog_cache[key]

    shared = {}
    for e in range(NUM_EXPERTS):
        for li, L in enumerate(experts[e]["layers"]):
            shared[f"w{e}_{li}"] = L["wb"]
            if L["fold"]:
                shared[f"cs{e}_{li}"] = L["cs"]
            if L["has_bias"]:
                shared[f"b{e}_{li}"] = L["bias_col"]
        if experts[e]["out_affine"]:
            shared[f"go{e}"] = np.ascontiguousarray(
                experts[e]["go"].reshape(8, P).T.astype(np.float32))
            shared[f"bto{e}"] = np.ascontiguousarray(
                experts[e]["bto"].reshape(8, P).T.astype(np.float32))

    in_maps = []
    for c in range(N_CORES):
        im = dict(shared)
        for e in range(NUM_EXPERTS):
            ids = chunks[e][c]
            xT = np.zeros((D_MODEL, caps[e]), dtype=BF)
            if len(ids):
                xT[:, :len(ids)] = x[ids].T.astype(BF)
            im[f"x{e}"] = xT
        in_maps.append(im)

    res = run_bass_kernel_spmd(nc, in_maps, list(range(N_CORES)))

    # ---- combine: weighted scatter-add ----
    out = np.zeros((B, D_MODEL), dtype=np.float32)
    for e in range(NUM_EXPERTS):
        for c in range(N_CORES):
            ids = chunks[e][c]
            if not len(ids):
                continue
            ye = res.results[c][f"y{e}"]  # [D_MODEL, cap] fp32
            out[ids] += gates[ids, e:e + 1] * ye[:, :len(ids)].T
    return out
